# revision 1
# baseline (speedup 1.0000x reference)
"""GAT layer kernel for Trainium2, 8 NeuronCores, edge/node-parallel.

Strategy (dst-sorted node sharding):
  - Sort edges by dst; partition nodes into 8 contiguous ranges with ~E/8
    edges each.  Each core owns its dst-range's nodes and all their in-edges.
  - Node phase (replicated): LayerNorm stats for all nodes via PE matmuls on a
    host-pretransposed h^T, packed finish on DVE/ACT -> eh, et; build a DRAM
    table T3[n] = [h[n] | 1.0 | eh[n] | et[n] | pad] (192 f32 = 768 B rows).
  - Edge phase per 128-dst block: er = tanh(LN(r)@w) from a host-pretransposed
    r^T stream (PE stats matmuls + batched row finish), dma_gather of T3 rows
    by src (768 B) and of the scalar region by dst (256 B), softmax without
    max-subtraction (e in [0,3) so exp is safe; exp(relu(x)) == max(1,exp(x))),
    scaled one-hot built in ONE tensor_scalar op, and a PSUM-accumulated
    matmul onehot^T @ [h|1] that yields feat and esum together.
  - Final: feat/esum, feat @ fc_w + b, row L2 normalize, DMA out.
"""

import os
import sys

sys.path.insert(0, "/opt/trn_rl_repo")

_PHASES = int(os.environ.get("KPHASES", "4"))

import numpy as np

import concourse.bacc as bacc
import concourse.bass as bass
import concourse.mybir as mybir
import concourse.tile as tile
from concourse.bass_interp import get_hw_module

F32 = mybir.dt.float32
I16 = mybir.dt.int16
AF = mybir.ActivationFunctionType
OP = mybir.AluOpType

N = 20000
E = 640000
D = 128
NCORES = 8
EPS = 1e-6
NPAD = 20480          # nodes padded to 40*512
NB_U = 21             # uniform blocks (of 128 dst nodes) per core
TROW = 192            # T3 table row: [h(128) | 1 | eh | et | pad] f32 (768B)
SCOFF = 128           # scalar region offset in T3 row
SCW = 64              # scalar region width (256B)


# ----------------------------------------------------------------- host prep
def _host_prep(h, r, src, dst, hn_a, hn_b, tn_a, tn_b, rn_a, rn_b,
               head_w, tail_w, rel_w, fc_w, fc_b):
    h = np.asarray(h, np.float32); r = np.asarray(r, np.float32)
    src = np.asarray(src, np.int32); dst = np.asarray(dst, np.int32)

    u_h = np.asarray(hn_a, np.float32) * np.asarray(head_w, np.float32)
    u_t = np.asarray(tn_a, np.float32) * np.asarray(tail_w, np.float32)
    u_r = np.asarray(rn_a, np.float32) * np.asarray(rel_w, np.float32)
    s_uh = float(u_h.sum()); s_ut = float(u_t.sum()); s_ur = float(u_r.sum())
    c_h = float((np.asarray(hn_b, np.float32) * head_w).sum())
    c_t = float((np.asarray(tn_b, np.float32) * tail_w).sum())
    c_r = float((np.asarray(rn_b, np.float32) * rel_w).sum())

    perm = np.argsort(dst, kind="stable")
    dst_s = dst[perm]; src_s = src[perm]

    counts = np.bincount(dst, minlength=N)
    cum = np.concatenate([[0], np.cumsum(counts)])

    # node range boundaries: ~E/8 edges each, capped at NB_U*128 nodes
    bounds = [0]
    for k in range(1, NCORES):
        n = int(np.searchsorted(cum, k * E / NCORES))
        n = max(bounds[-1] + 1, min(n, bounds[-1] + NB_U * 128))
        n = max(n, N - (NCORES - k) * NB_U * 128)   # leave room for the rest
        bounds.append(n)
    bounds.append(N)

    # T_B: max tiles over every (core, block)
    t_b = 1
    for k in range(NCORES):
        nlo, nhi = bounds[k], bounds[k + 1]
        for b0 in range(nlo, nhi, 128):
            cnt = int(cum[min(b0 + 128, nhi)] - cum[b0])
            t_b = max(t_b, (cnt + 127) // 128)
    e_blk = t_b * 128
    s_b = e_blk // 16
    ep = NB_U * e_blk

    # replicated tensors
    h_pad = np.empty((NPAD, D), np.float32)
    h_pad[:N] = h; h_pad[N:] = h[0]
    hT = np.ascontiguousarray(h_pad.T)
    iota = np.broadcast_to(np.arange(128, dtype=np.float32), (128, 128)).copy()
    ident = np.eye(128, dtype=np.float32)
    wn = np.zeros((128, 4), np.float32)
    wn[:, 0] = 1.0; wn[:, 1] = u_h; wn[:, 2] = u_t
    wr = np.zeros((128, 2), np.float32)
    wr[:, 0] = 1.0; wr[:, 1] = u_r
    fcw = np.ascontiguousarray(np.asarray(fc_w, np.float32))
    fcb = np.broadcast_to(np.asarray(fc_b, np.float32), (128, 128)).copy()
    consts = np.zeros((128, 8), np.float32)
    consts[:, 0] = s_uh; consts[:, 1] = s_ut; consts[:, 2] = s_ur
    consts[:, 3] = c_h; consts[:, 4] = c_t; consts[:, 5] = c_r

    rep = {"hT": hT, "h_nat": h_pad, "iota": iota, "ident": ident,
           "wn": wn, "wr": wr, "fcw": fcw, "fcb": fcb, "consts": consts}

    in_maps = []
    for k in range(NCORES):
        nlo, nhi = bounds[k], bounds[k + 1]
        nb = (nhi - nlo + 127) // 128
        # per-slot arrays, one row of NB_U*e_blk slots
        src16 = np.zeros((NB_U, e_blk), np.int16)
        dst16 = np.zeros((NB_U, e_blk), np.int16)
        dstl = np.zeros((NB_U, e_blk), np.float32)
        valid = np.zeros((NB_U, e_blk), np.float32)
        rcol = np.zeros((NB_U, e_blk), np.int64)
        for b in range(nb):
            b0 = nlo + 128 * b
            e0, e1 = int(cum[b0]), int(cum[min(b0 + 128, nhi)])
            cnt = e1 - e0
            src16[b, :cnt] = src_s[e0:e1]
            dst16[b, :cnt] = dst_s[e0:e1]
            dstl[b, :cnt] = (dst_s[e0:e1] - b0).astype(np.float32)
            valid[b, :cnt] = 1.0
            rcol[b, :cnt] = perm[e0:e1]
        # rT: [128, ep], column (b*e_blk + j) = r[rcol]
        rT = np.ascontiguousarray(r[rcol.reshape(-1)].T)
        # idx tensors: per block wrap 16, replicate x8 -> [128, NB_U*s_b]
        def wrap16(a):
            blk = a.reshape(NB_U, s_b, 16).transpose(0, 2, 1)  # [NB_U,16,s_b]
            out = np.tile(blk, (1, 8, 1))                       # [NB_U,128,s_b]
            return np.ascontiguousarray(out.transpose(1, 0, 2).reshape(128, NB_U * s_b))
        idx_src = wrap16(src16)
        idx_dst = wrap16(dst16)
        # packed [128, NB_U*t_b]: [p, b*t_b+t] = slot j=128t+p
        def pk(a):
            x = a.reshape(NB_U, t_b, 128).transpose(2, 0, 1)   # [128, NB_U, t_b]
            return np.ascontiguousarray(x.reshape(128, NB_U * t_b))
        in_maps.append(dict(rep, rT=rT, idx_src=idx_src, idx_dst=idx_dst,
                            dstl=pk(dstl), valid=pk(valid)))
    meta = dict(t_b=t_b, e_blk=e_blk, s_b=s_b, ep=ep, bounds=bounds)
    return in_maps, meta


# ------------------------------------------------------------ device program
def build_program(t_b, loop_k=1, for_hw=True):
    e_blk = t_b * 128
    s_b = e_blk // 16
    ep = NB_U * e_blk
    nc = bacc.Bacc("TRN2", target_bir_lowering=False, debug=False,
                   enable_asserts=False, num_devices=NCORES if for_hw else 1)

    dt_rT = nc.dram_tensor("rT", [128, ep], F32, kind="ExternalInput")
    dt_hT = nc.dram_tensor("hT", [128, NPAD], F32, kind="ExternalInput")
    dt_hn = nc.dram_tensor("h_nat", [NPAD, D], F32, kind="ExternalInput")
    dt_isrc = nc.dram_tensor("idx_src", [128, NB_U * s_b], I16, kind="ExternalInput")
    dt_idst = nc.dram_tensor("idx_dst", [128, NB_U * s_b], I16, kind="ExternalInput")
    dt_dstl = nc.dram_tensor("dstl", [128, NB_U * t_b], F32, kind="ExternalInput")
    dt_valid = nc.dram_tensor("valid", [128, NB_U * t_b], F32, kind="ExternalInput")
    dt_iota = nc.dram_tensor("iota", [128, 128], F32, kind="ExternalInput")
    dt_ident = nc.dram_tensor("ident", [128, 128], F32, kind="ExternalInput")
    dt_wn = nc.dram_tensor("wn", [128, 4], F32, kind="ExternalInput")
    dt_wr = nc.dram_tensor("wr", [128, 2], F32, kind="ExternalInput")
    dt_fcw = nc.dram_tensor("fcw", [128, 128], F32, kind="ExternalInput")
    dt_fcb = nc.dram_tensor("fcb", [128, 128], F32, kind="ExternalInput")
    dt_consts = nc.dram_tensor("consts", [128, 8], F32, kind="ExternalInput")
    dt_out = nc.dram_tensor("out", [NB_U * 128, 128], F32, kind="ExternalOutput")
    dt_T3 = nc.dram_tensor("T3", [NPAD, TROW], F32, kind="ExternalOutput")

    NG = NPAD // 512          # node-phase groups
    NPK = NPAD // 128         # packed node cols
    EPK = NB_U * t_b          # packed edge cols
    GE = (e_blk + 511) // 512  # stats groups per block

    with tile.TileContext(nc) as tc:
        with tc.tile_pool(name="const", bufs=1) as cpool:
            iota_sb = cpool.tile([128, 128], F32)
            nc.sync.dma_start(out=iota_sb[:], in_=dt_iota.ap())
            ident_sb = cpool.tile([128, 128], F32)
            nc.sync.dma_start(out=ident_sb[:], in_=dt_ident.ap())
            wn_sb = cpool.tile([128, 4], F32)
            nc.sync.dma_start(out=wn_sb[:], in_=dt_wn.ap())
            wr_sb = cpool.tile([128, 2], F32)
            nc.sync.dma_start(out=wr_sb[:], in_=dt_wr.ap())
            fcw_sb = cpool.tile([128, 128], F32)
            nc.sync.dma_start(out=fcw_sb[:], in_=dt_fcw.ap())
            fcb_sb = cpool.tile([128, 128], F32)
            nc.sync.dma_start(out=fcb_sb[:], in_=dt_fcb.ap())
            cst = cpool.tile([128, 8], F32)
            nc.sync.dma_start(out=cst[:], in_=dt_consts.ap())
            isrc_sb = cpool.tile([128, NB_U * s_b], I16)
            nc.sync.dma_start(out=isrc_sb[:], in_=dt_isrc.ap())
            idst_sb = cpool.tile([128, NB_U * s_b], I16)
            nc.sync.dma_start(out=idst_sb[:], in_=dt_idst.ap())
            dstl_sb = cpool.tile([128, NB_U * t_b], F32)
            nc.sync.dma_start(out=dstl_sb[:], in_=dt_dstl.ap())
            valid_sb = cpool.tile([128, NB_U * t_b], F32)
            nc.sync.dma_start(out=valid_sb[:], in_=dt_valid.ap())

            def loop_body():
                # ======================= node phase: stats =======================
                # per 128-node tile: mm(lhsT=hT_slice [D,128n], rhs=wn [D,3])
                # -> psum [128n, 3] already packed; s2 via squared lhsT, N=1.
                with tc.tile_pool(name="nstat", bufs=1) as spool, \
                     tc.tile_pool(name="nwork", bufs=3) as wpool, \
                     tc.tile_pool(name="npsum", bufs=4, space="PSUM") as pp:
                    spk = spool.tile([128, NPK, 4], F32)
                    ehp = spool.tile([128, NPK], F32)
                    etp = spool.tile([128, NPK], F32)
                    for g in range(NG):
                        hTg = wpool.tile([128, 512], F32, tag="hTg")
                        nc.sync.dma_start(out=hTg[:], in_=dt_hT.ap()[:, 512 * g:512 * (g + 1)])
                        psS = pp.tile([128, 16], F32, tag="psS")
                        for c in range(4):
                            nc.tensor.matmul(psS[:, 4 * c:4 * c + 3],
                                             hTg[:, 128 * c:128 * (c + 1)],
                                             wn_sb[:, 0:3], start=True, stop=True)
                        nc.scalar.activation(out=hTg[:], in_=hTg[:], func=AF.Square)
                        for c in range(4):
                            nc.tensor.matmul(psS[:, 4 * c + 3:4 * c + 4],
                                             hTg[:, 128 * c:128 * (c + 1)],
                                             wn_sb[:, 0:1], start=True, stop=True)
                        nc.scalar.activation(out=spk[:, 4 * g:4 * (g + 1), :], in_=psS[:], func=AF.Copy)
                    # batched finish -> eh, et (strided stat views)
                    s1p = spk[:, :, 0]; suh = spk[:, :, 1]
                    sut = spk[:, :, 2]; s2p = spk[:, :, 3]
                    mu = spool.tile([128, NPK], F32)
                    nc.vector.tensor_scalar_mul(out=mu[:], in0=s1p, scalar1=1.0 / 128.0)
                    t0 = spool.tile([128, NPK], F32)
                    nc.vector.tensor_mul(out=t0[:], in0=mu[:], in1=mu[:])
                    nc.vector.tensor_scalar_mul(out=t0[:], in0=t0[:], scalar1=-128.0)
                    nc.vector.tensor_add(out=t0[:], in0=t0[:], in1=s2p)
                    rstd = spool.tile([128, NPK], F32)
                    nc.scalar.activation(out=rstd[:], in_=t0[:], func=AF.Sqrt, scale=1.0 / 127.0)
                    nc.vector.tensor_scalar_add(out=rstd[:], in0=rstd[:], scalar1=EPS)
                    nc.vector.reciprocal(out=rstd[:], in_=rstd[:])
                    for su, sidx, cidx, dest in ((suh, 0, 3, ehp), (sut, 1, 4, etp)):
                        m1 = spool.tile([128, NPK], F32, tag="m1")
                        nc.vector.tensor_scalar_mul(out=m1[:], in0=mu[:], scalar1=cst[:, sidx:sidx + 1])
                        nc.vector.tensor_sub(out=m1[:], in0=su, in1=m1[:])
                        nc.vector.tensor_mul(out=m1[:], in0=m1[:], in1=rstd[:])
                        nc.vector.tensor_scalar_add(out=m1[:], in0=m1[:], scalar1=cst[:, cidx:cidx + 1])
                        nc.scalar.activation(out=dest[:], in_=m1[:], func=AF.Tanh)
                    if _PHASES == 1:
                        nc.sync.dma_start(out=dt_out.ap()[0:128, 0:min(NPK, 128)],
                                          in_=ehp[:, 0:min(NPK, 128)])
                        return
                    # ====================== T3 table build ======================
                    with tc.tile_pool(name="tbld", bufs=3) as tb_pool:
                        for g in range(NG):
                            tb = tb_pool.tile([128, 4, TROW], F32, tag="tb")
                            nc.sync.dma_start(
                                out=tb[:, :, 0:D],
                                in_=dt_hn.ap()[512 * g:512 * (g + 1), :]
                                    .rearrange("(c p) d -> p c d", p=128))
                            nc.vector.memset(tb[:, :, SCOFF:SCOFF + 1], 1.0)
                            nc.vector.tensor_copy(out=tb[:, :, SCOFF + 1], in_=ehp[:, 4 * g:4 * (g + 1)])
                            nc.vector.tensor_copy(out=tb[:, :, SCOFF + 2], in_=etp[:, 4 * g:4 * (g + 1)])
                            nc.vector.memset(tb[:, :, SCOFF + 3:TROW], 0.0)
                            nc.sync.dma_start(
                                out=dt_T3.ap()[512 * g:512 * (g + 1), :]
                                    .rearrange("(c p) w -> p c w", p=128),
                                in_=tb[:, :, :])

                # ===================== edge phase 1: er =========================
                with tc.tile_pool(name="estat", bufs=1) as espool:
                    epk3 = espool.tile([128, EPK, 3], F32)
                    erp = espool.tile([128, EPK], F32)
                    with tc.tile_pool(name="ework", bufs=2) as ewpool, \
                         tc.tile_pool(name="epsum", bufs=2, space="PSUM") as epp:
                        for b in range(NB_U):
                            rTb = ewpool.tile([128, e_blk], F32, tag="rTb")
                            nc.sync.dma_start(out=rTb[:], in_=dt_rT.ap()[:, b * e_blk:(b + 1) * e_blk])
                            psE = epp.tile([128, 3 * t_b], F32, tag="psE")
                            for t in range(t_b):
                                nc.tensor.matmul(psE[:, 3 * t:3 * t + 2],
                                                 rTb[:, 128 * t:128 * (t + 1)],
                                                 wr_sb[:], start=True, stop=True)
                            nc.scalar.activation(out=rTb[:], in_=rTb[:], func=AF.Square)
                            for t in range(t_b):
                                nc.tensor.matmul(psE[:, 3 * t + 2:3 * t + 3],
                                                 rTb[:, 128 * t:128 * (t + 1)],
                                                 wr_sb[:, 0:1], start=True, stop=True)
                            nc.scalar.activation(out=epk3[:, b * t_b:(b + 1) * t_b, :],
                                                 in_=psE[:], func=AF.Copy)
                    # batched er finish (strided stat views)
                    s1e = epk3[:, :, 0]; sue = epk3[:, :, 1]; s2e = epk3[:, :, 2]
                    mu = espool.tile([128, EPK], F32)
                    nc.vector.tensor_scalar_mul(out=mu[:], in0=s1e, scalar1=1.0 / 128.0)
                    t0 = espool.tile([128, EPK], F32)
                    nc.vector.tensor_mul(out=t0[:], in0=mu[:], in1=mu[:])
                    nc.vector.tensor_scalar_mul(out=t0[:], in0=t0[:], scalar1=-128.0)
                    nc.vector.tensor_add(out=t0[:], in0=t0[:], in1=s2e)
                    rstd = espool.tile([128, EPK], F32)
                    nc.scalar.activation(out=rstd[:], in_=t0[:], func=AF.Sqrt, scale=1.0 / 127.0)
                    nc.vector.tensor_scalar_add(out=rstd[:], in0=rstd[:], scalar1=EPS)
                    nc.vector.reciprocal(out=rstd[:], in_=rstd[:])
                    m1 = espool.tile([128, EPK], F32)
                    nc.vector.tensor_scalar_mul(out=m1[:], in0=mu[:], scalar1=cst[:, 2:3])
                    nc.vector.tensor_sub(out=m1[:], in0=sue, in1=m1[:])
                    nc.vector.tensor_mul(out=m1[:], in0=m1[:], in1=rstd[:])
                    nc.vector.tensor_scalar_add(out=m1[:], in0=m1[:], scalar1=cst[:, 5:6])
                    nc.scalar.activation(out=erp[:], in_=m1[:], func=AF.Tanh)
                    if _PHASES == 2:
                        nc.sync.dma_start(out=dt_out.ap()[0:128, 0:min(EPK, 128)],
                                          in_=erp[:, 0:min(EPK, 128)])
                        return

                    # ================= edge phase 2: gather + feat ==============
                    with tc.tile_pool(name="gwork", bufs=2) as gpool, \
                         tc.tile_pool(name="feat", bufs=1) as fpool, \
                         tc.tile_pool(name="fpsum", bufs=2, space="PSUM") as fpp:
                        featst = fpool.tile([128, NB_U * 129], F32)
                        for b in range(NB_U):
                            tg = gpool.tile([128, t_b, TROW], F32, tag="tg")
                            nc.gpsimd.dma_gather(
                                out_ap=tg[:, :, :], in_ap=dt_T3.ap(),
                                idxs_ap=isrc_sb[:, b * s_b:(b + 1) * s_b],
                                num_idxs=e_blk, num_idxs_reg=e_blk, elem_size=TROW,
                                single_packet=False)
                            sc = gpool.tile([128, t_b, SCW], F32, tag="sc")
                            nc.gpsimd.dma_gather(
                                out_ap=sc[:, :, :], in_ap=dt_T3.ap()[:, SCOFF:TROW],
                                idxs_ap=idst_sb[:, b * s_b:(b + 1) * s_b],
                                num_idxs=e_blk, num_idxs_reg=e_blk,
                                elem_size=SCW, elem_step=TROW, single_packet=False)
                            # exe = max(1, exp(eh_src + et_dst + er)) * valid
                            ex = gpool.tile([128, t_b], F32, tag="ex")
                            nc.vector.tensor_tensor(out=ex[:], in0=tg[:, :, SCOFF + 1],
                                                    in1=sc[:, :, 2], op=OP.add)
                            nc.vector.tensor_add(out=ex[:], in0=ex[:],
                                                 in1=erp[:, b * t_b:(b + 1) * t_b])
                            nc.scalar.activation(out=ex[:], in_=ex[:], func=AF.Exp)
                            nc.vector.tensor_scalar(out=ex[:], in0=ex[:],
                                                    scalar1=1.0, scalar2=None,
                                                    op0=OP.max)
                            nc.vector.tensor_mul(out=ex[:], in0=ex[:],
                                                 in1=valid_sb[:, b * t_b:(b + 1) * t_b])
                            psF = fpp.tile([128, 129], F32, tag="psF")
                            for t in range(t_b):
                                oh = gpool.tile([128, 128], F32, tag="oh")
                                nc.vector.tensor_scalar(
                                    out=oh[:], in0=iota_sb[:],
                                    scalar1=dstl_sb[:, b * t_b + t:b * t_b + t + 1],
                                    scalar2=ex[:, t:t + 1],
                                    op0=OP.is_equal, op1=OP.mult)
                                nc.tensor.matmul(psF[:], oh[:], tg[:, t, 0:129],
                                                 start=(t == 0), stop=(t == t_b - 1))
                            nc.scalar.activation(out=featst[:, b * 129:(b + 1) * 129],
                                                 in_=psF[:], func=AF.Copy)
                        if _PHASES == 3:
                            nc.sync.dma_start(out=dt_out.ap()[0:128, 0:128],
                                              in_=featst[:, 0:128])
                            return
                        # =================== final per block ====================
                        with tc.tile_pool(name="fin", bufs=2) as npool, \
                             tc.tile_pool(name="finps", bufs=2, space="PSUM") as npp:
                            for b in range(NB_U):
                                rs = npool.tile([128, 1], F32, tag="rs")
                                nc.vector.tensor_scalar(out=rs[:], in0=featst[:, b * 129 + 128:b * 129 + 129],
                                                        scalar1=1e-30, scalar2=None, op0=OP.max)
                                nc.vector.reciprocal(out=rs[:], in_=rs[:])
                                fs = npool.tile([128, 128], F32, tag="fs")
                                nc.vector.tensor_scalar_mul(
                                    out=fs[:], in0=featst[:, b * 129:b * 129 + 128], scalar1=rs[:])
                                if _PHASES == 5:
                                    nc.sync.dma_start(out=dt_out.ap()[b * 128:(b + 1) * 128, :], in_=fs[:])
                                    continue
                                psT = npp.tile([128, 128], F32, tag="psT")
                                nc.tensor.transpose(psT[:], fs[:], ident_sb[:])
                                fT = npool.tile([128, 128], F32, tag="fT")
                                nc.scalar.activation(out=fT[:], in_=psT[:], func=AF.Copy)
                                if _PHASES == 6:
                                    nc.sync.dma_start(out=dt_out.ap()[b * 128:(b + 1) * 128, :], in_=fT[:])
                                    continue
                                psO = npp.tile([128, 128], F32, tag="psO")
                                nc.tensor.matmul(psO[:], fT[:], fcw_sb[:], start=True, stop=True)
                                ob = npool.tile([128, 128], F32, tag="ob")
                                nc.vector.tensor_add(out=ob[:], in0=psO[:], in1=fcb_sb[:])
                                if _PHASES == 7:
                                    nc.sync.dma_start(out=dt_out.ap()[b * 128:(b + 1) * 128, :], in_=ob[:])
                                    continue
                                scr = npool.tile([128, 128], F32, tag="scr")
                                nrm = npool.tile([128, 1], F32, tag="nrm")
                                nc.vector.tensor_mul(out=scr[:], in0=ob[:], in1=ob[:])
                                nc.vector.reduce_sum(out=nrm[:], in_=scr[:],
                                                     axis=mybir.AxisListType.X)
                                if _PHASES == 8:
                                    nc.sync.dma_start(out=dt_out.ap()[b * 128:(b + 1) * 128, :], in_=scr[:])
                                    continue
                                nc.scalar.activation(out=nrm[:], in_=nrm[:], func=AF.Sqrt)
                                if _PHASES == 9:
                                    nc.sync.dma_start(out=dt_out.ap()[b * 128:(b + 1) * 128, 0:1], in_=nrm[:])
                                    continue
                                nc.vector.tensor_scalar(out=nrm[:], in0=nrm[:],
                                                        scalar1=1e-12, scalar2=None, op0=OP.max)
                                nc.vector.reciprocal(out=nrm[:], in_=nrm[:])
                                nc.vector.tensor_scalar_mul(out=ob[:], in0=ob[:], scalar1=nrm[:])
                                nc.sync.dma_start(out=dt_out.ap()[b * 128:(b + 1) * 128, :], in_=ob[:])

            if loop_k == 1:
                loop_body()
            else:
                with tc.For_i(0, loop_k, 1):
                    loop_body()

    nc.compile()
    if for_hw:
        nc.m = get_hw_module(nc.m)
    return nc


# ------------------------------------------------------------------- runner
class Runner:
    def __init__(self, nc, n_cores=NCORES):
        import jax
        from concourse.bass2jax import (_bass_exec_p, partition_id_tensor,
                                        install_neuronx_cc_hook)
        from jax.sharding import Mesh, PartitionSpec, NamedSharding
        from jax.experimental.shard_map import shard_map
        install_neuronx_cc_hook()
        self.jax = jax
        self.n_cores = n_cores
        pname = nc.partition_id_tensor.name if nc.partition_id_tensor else None
        in_names, out_names, out_avals = [], [], []
        for alloc in nc.m.functions[0].allocations:
            if not isinstance(alloc, mybir.MemoryLocationSet):
                continue
            name = alloc.memorylocations[0].name
            if alloc.kind == "ExternalInput":
                if name != pname:
                    in_names.append(name)
            elif alloc.kind == "ExternalOutput":
                out_names.append(name)
                out_avals.append(jax.core.ShapedArray(
                    tuple(alloc.tensor_shape), mybir.dt.np(alloc.dtype)))
        self.in_names, self.out_names, self.out_avals = in_names, out_names, out_avals
        n_params = len(in_names)
        all_in = list(in_names) + list(out_names)
        if pname is not None:
            all_in.append(pname)

        def _body(*args):
            operands = list(args)
            if pname is not None:
                operands.append(partition_id_tensor())
            return tuple(_bass_exec_p.bind(
                *operands, out_avals=tuple(out_avals), in_names=tuple(all_in),
                out_names=tuple(out_names), lowering_input_output_aliases=(),
                sim_require_finite=True, sim_require_nnan=True, nc=nc))

        devices = jax.devices()[:n_cores]
        self.mesh = Mesh(np.asarray(devices), ("core",))
        self.sharding = NamedSharding(self.mesh, PartitionSpec("core"))
        donate = tuple(range(n_params, n_params + len(out_names)))
        self.fn = jax.jit(shard_map(
            _body, mesh=self.mesh,
            in_specs=(PartitionSpec("core"),) * (n_params + len(out_names)),
            out_specs=(PartitionSpec("core"),) * len(out_names),
            check_rep=False), donate_argnums=donate, keep_unused=True)

    def put_inputs(self, in_maps):
        return [self.jax.device_put(
            np.concatenate([np.asarray(in_maps[c][nm]) for c in range(self.n_cores)], axis=0),
            self.sharding) for nm in self.in_names]

    def put_zeros(self):
        return [self.jax.device_put(
            np.zeros((self.n_cores * a.shape[0], *a.shape[1:]), a.dtype), self.sharding)
            for a in self.out_avals]

    def run(self, dev_in, dev_zeros):
        outs = self.fn(*dev_in, *dev_zeros)
        self.jax.block_until_ready(outs)
        return outs

    def unpack(self, outs):
        return [{nm: np.asarray(outs[i]).reshape(self.n_cores, *self.out_avals[i].shape)[c]
                 for i, nm in enumerate(self.out_names)} for c in range(self.n_cores)]


_CACHE = {}


def _get_runner(t_b, loop_k=1):
    key = (t_b, loop_k)
    if key not in _CACHE:
        nc = build_program(t_b, loop_k)
        _CACHE[key] = Runner(nc)
    return _CACHE[key]


def kernel(**inputs):
    in_maps, meta = _host_prep(**inputs)
    r = _get_runner(meta["t_b"], 1)
    dev = r.put_inputs(in_maps)
    res = r.unpack(r.run(dev, r.put_zeros()))
    bounds = meta["bounds"]
    out = np.empty((N, D), np.float32)
    for k in range(NCORES):
        nlo, nhi = bounds[k], bounds[k + 1]
        out[nlo:nhi] = res[k]["out"][:nhi - nlo]
    return out



# revision 2
# speedup vs baseline: 4.3419x; 4.3419x over previous
"""GAT layer kernel for Trainium2, 8 NeuronCores — v3.

v2 -> v3 (from NTFF trace of v2):
  - ACT table loads (1283ns each, 120 of them) killed by batching all
    Sqrt/Tanh finishes across blocks; steady-state ACT uses only
    {Square, Exp, Copy, Tanh} = one table set.
  - Per-tile one-hot tensor_scalar (1.4us each on f16 w/ AP scalars!)
    replaced by whole-block broadcast tensor_tensor is_equal (plain one-hot)
    + one whole-block scale of the gathered rows by ex.
  - er stats + etd matmuls moved to loop A (independent of gathers) so they
    pipeline under the serial gather-DGE stream; gather tiles are consumed
    immediately (tgs scale) so DGE runs back-to-back.
  - Final normalize batched across blocks; single output DMA.
"""

import os
import sys

sys.path.insert(0, "/opt/trn_rl_repo")

import numpy as np

import concourse.bacc as bacc
import concourse.bass as bass
import concourse.mybir as mybir
import concourse.tile as tile
from concourse.bass_interp import get_hw_module

F32 = mybir.dt.float32
F16 = mybir.dt.float16
I16 = mybir.dt.int16
AF = mybir.ActivationFunctionType
OP = mybir.AluOpType

N = 20000
E = 640000
D = 128
NCORES = 8
EPS = 1e-6
NPAD = 20480
NG = NPAD // 512
NPK = NPAD // 128
NB_U = 21
NB_NODES = NB_U * 128
TROW = 256


# ----------------------------------------------------------------- host prep
def _host_prep(h, r, src, dst, hn_a, hn_b, tn_a, tn_b, rn_a, rn_b,
               head_w, tail_w, rel_w, fc_w, fc_b):
    h = np.asarray(h, np.float32); r = np.asarray(r, np.float32)
    src = np.asarray(src, np.int32); dst = np.asarray(dst, np.int32)

    u_h = np.asarray(hn_a, np.float32) * np.asarray(head_w, np.float32)
    u_t = np.asarray(tn_a, np.float32) * np.asarray(tail_w, np.float32)
    u_r = np.asarray(rn_a, np.float32) * np.asarray(rel_w, np.float32)
    s_uh = float(u_h.sum()); s_ut = float(u_t.sum()); s_ur = float(u_r.sum())
    c_h = float((np.asarray(hn_b, np.float32) * head_w).sum())
    c_t = float((np.asarray(tn_b, np.float32) * tail_w).sum())
    c_r = float((np.asarray(rn_b, np.float32) * rel_w).sum())

    perm = np.argsort(dst, kind="stable")
    dst_s = dst[perm]; src_s = src[perm]

    counts = np.bincount(dst, minlength=N)
    cum = np.concatenate([[0], np.cumsum(counts)])

    t_b = 1
    for b0 in range(0, N, 128):
        cnt = int(cum[min(b0 + 128, N)] - cum[b0])
        t_b = max(t_b, (cnt + 127) // 128)
    e_blk = t_b * 128
    s_b = e_blk // 16
    ep = NB_U * e_blk

    h_pad = np.empty((NPAD, D), np.float32)
    h_pad[:N] = h; h_pad[N:] = h[0]
    h16 = h_pad.astype(np.float16)
    hT16_g = np.ascontiguousarray(h16.T)

    iota16 = np.broadcast_to(np.arange(128, dtype=np.float16), (128, 128)).copy()
    ident = np.eye(128, dtype=np.float16)
    wn16 = np.zeros((128, 3), np.float16)
    wn16[:, 0] = 1.0; wn16[:, 1] = u_h.astype(np.float16); wn16[:, 2] = u_t.astype(np.float16)
    wr16 = np.zeros((128, 2), np.float16)
    wr16[:, 0] = 1.0; wr16[:, 1] = u_r.astype(np.float16)
    ones16 = np.ones((128, 1), np.float16)
    fcw16 = np.asarray(fc_w, np.float32).astype(np.float16)
    fcb = np.broadcast_to(np.asarray(fc_b, np.float32), (128, 128)).copy()
    consts = np.zeros((128, 8), np.float32)
    consts[:, 0] = s_uh; consts[:, 1] = s_ut; consts[:, 2] = s_ur
    consts[:, 3] = c_h; consts[:, 4] = c_t; consts[:, 5] = c_r

    rep = {"iota16": iota16, "ident": ident, "wn16": wn16, "wr16": wr16,
           "ones16": ones16, "fcw16": fcw16, "fcb": fcb, "consts": consts}

    vidx = np.arange(128, dtype=np.float32)

    in_maps = []
    for k in range(NCORES):
        o = k * NB_NODES
        nlo, nhi = o, min(o + NB_NODES, N)
        src16 = np.zeros((NB_U, e_blk), np.int16)
        dstl = np.full((NB_U, e_blk), -1.0, np.float32)
        rcol = np.zeros((NB_U, e_blk), np.int64)
        for b in range(NB_U):
            b0 = nlo + 128 * b
            if b0 >= nhi:
                break
            e0, e1 = int(cum[b0]), int(cum[min(b0 + 128, nhi)])
            cnt = e1 - e0
            src16[b, :cnt] = ((src_s[e0:e1] - o) % NPAD).astype(np.int16)
            dstl[b, :cnt] = (dst_s[e0:e1] - b0).astype(np.float32)
            rcol[b, :cnt] = perm[e0:e1]
        rT16 = np.ascontiguousarray(r[rcol.reshape(-1)].T).astype(np.float16)

        def wrap16(a):
            blk = a.reshape(NB_U, s_b, 16).transpose(0, 2, 1)
            out = np.tile(blk, (1, 8, 1))
            return np.ascontiguousarray(out.transpose(1, 0, 2).reshape(128, NB_U * s_b))
        idx_src = wrap16(src16)

        def pk(a):
            x = a.reshape(NB_U, t_b, 128).transpose(2, 0, 1)
            return np.ascontiguousarray(x.reshape(128, NB_U * t_b))
        dstl_pk = pk(dstl)

        dl3 = dstl.reshape(NB_U, t_b * 128)
        oht = (dl3[None, :, :] == vidx[:, None, None])
        oht = np.ascontiguousarray(oht.reshape(128, NB_U * e_blk)).astype(np.float16)

        h_ord = np.roll(h16, -o, axis=0)
        hT16 = np.roll(hT16_g, -o, axis=1)
        h16g = np.ascontiguousarray(
            h_ord.reshape(NG, 4, 128, D).transpose(2, 0, 1, 3).reshape(128, NG * 512))

        in_maps.append(dict(rep, rT16=rT16, idx_src=idx_src, dstl=dstl_pk,
                            oht=oht, hT16=hT16, h16g=h16g))
    meta = dict(t_b=t_b, e_blk=e_blk, s_b=s_b, ep=ep)
    return in_maps, meta


# ------------------------------------------------------------ device program
def build_program(t_b, loop_k=1, for_hw=True):
    e_blk = t_b * 128
    s_b = e_blk // 16
    ep = NB_U * e_blk
    EPK = NB_U * t_b
    nc = bacc.Bacc("TRN2", target_bir_lowering=False, debug=False,
                   enable_asserts=False, num_devices=NCORES if for_hw else 1)

    dt_rT = nc.dram_tensor("rT16", [128, ep], F16, kind="ExternalInput")
    dt_hT = nc.dram_tensor("hT16", [128, NPAD], F16, kind="ExternalInput")
    dt_h16g = nc.dram_tensor("h16g", [128, NG * 512], F16, kind="ExternalInput")
    dt_isrc = nc.dram_tensor("idx_src", [128, NB_U * s_b], I16, kind="ExternalInput")
    dt_dstl = nc.dram_tensor("dstl", [128, NB_U * t_b], F32, kind="ExternalInput")
    dt_oht = nc.dram_tensor("oht", [128, NB_U * e_blk], F16, kind="ExternalInput")
    dt_iota = nc.dram_tensor("iota16", [128, 128], F16, kind="ExternalInput")
    dt_ident = nc.dram_tensor("ident", [128, 128], F16, kind="ExternalInput")
    dt_wn = nc.dram_tensor("wn16", [128, 3], F16, kind="ExternalInput")
    dt_wr = nc.dram_tensor("wr16", [128, 2], F16, kind="ExternalInput")
    dt_ones = nc.dram_tensor("ones16", [128, 1], F16, kind="ExternalInput")
    dt_fcw = nc.dram_tensor("fcw16", [128, 128], F16, kind="ExternalInput")
    dt_fcb = nc.dram_tensor("fcb", [128, 128], F32, kind="ExternalInput")
    dt_consts = nc.dram_tensor("consts", [128, 8], F32, kind="ExternalInput")
    dt_out = nc.dram_tensor("out", [NB_U * 128, 128], F32, kind="ExternalOutput")
    dt_T3 = nc.dram_tensor("T3", [NPAD, TROW], F16, kind="ExternalOutput")

    with tile.TileContext(nc) as tc:
        with tc.tile_pool(name="const", bufs=1) as cpool:
            iota_sb = cpool.tile([128, 128], F16)
            nc.sync.dma_start(out=iota_sb[:], in_=dt_iota.ap())
            ident_sb = cpool.tile([128, 128], F16)
            nc.sync.dma_start(out=ident_sb[:], in_=dt_ident.ap())
            wn_sb = cpool.tile([128, 3], F16)
            nc.sync.dma_start(out=wn_sb[:], in_=dt_wn.ap())
            wr_sb = cpool.tile([128, 2], F16)
            nc.sync.dma_start(out=wr_sb[:], in_=dt_wr.ap())
            ones_sb = cpool.tile([128, 1], F16)
            nc.sync.dma_start(out=ones_sb[:], in_=dt_ones.ap())
            fcw_sb = cpool.tile([128, 128], F16)
            nc.sync.dma_start(out=fcw_sb[:], in_=dt_fcw.ap())
            fcb_sb = cpool.tile([128, 128], F32)
            nc.sync.dma_start(out=fcb_sb[:], in_=dt_fcb.ap())
            cst = cpool.tile([128, 8], F32)
            nc.sync.dma_start(out=cst[:], in_=dt_consts.ap())
            isrc_sb = cpool.tile([128, NB_U * s_b], I16)
            nc.sync.dma_start(out=isrc_sb[:], in_=dt_isrc.ap())
            dstl_sb = cpool.tile([128, NB_U * t_b], F32)
            nc.sync.dma_start(out=dstl_sb[:], in_=dt_dstl.ap())

            def loop_body():
                with tc.tile_pool(name="stage", bufs=1) as st:
                    et16 = st.tile([128, NPK], F16)
                    eh16 = st.tile([128, NPK], F16)
                    sNA = st.tile([128, NG, 16], F32)
                    s1A = st.tile([128, EPK], F32)
                    suA = st.tile([128, EPK], F32)
                    s2A = st.tile([128, EPK], F32)
                    etdA = st.tile([128, EPK], F32)
                    erpA = st.tile([128, EPK], F32)
                    obA = st.tile([128, NB_U, 128], F16)
                    fstA = st.tile([128, NB_U, 129], F32)

                    # ---------- node stats ----------
                    with tc.tile_pool(name="nwork", bufs=3) as wpool, \
                         tc.tile_pool(name="npsum", bufs=3, space="PSUM") as pp:
                        for g in range(NG):
                            hTg = wpool.tile([128, 512], F16, tag="hTg")
                            nc.sync.dma_start(out=hTg[:], in_=dt_hT.ap()[:, 512 * g:512 * (g + 1)])
                            hTg2 = wpool.tile([128, 512], F16, tag="hTg2")
                            nc.scalar.activation(out=hTg2[:], in_=hTg[:], func=AF.Square)
                            psN = pp.tile([128, 16], F32, tag="psN")
                            for c in range(4):
                                nc.tensor.matmul(psN[:, 3 * c:3 * c + 3],
                                                 hTg[:, 128 * c:128 * (c + 1)],
                                                 wn_sb[:], start=True, stop=True)
                                nc.tensor.matmul(psN[:, 12 + c:13 + c],
                                                 hTg2[:, 128 * c:128 * (c + 1)],
                                                 ones_sb[:], start=True, stop=True)
                            nc.vector.tensor_copy(out=sNA[:, g, :], in_=psN[:])

                    # ---------- T3 h-part + scalar part (direct DMAs) ----------
                    with tc.tile_pool(name="tbld", bufs=1) as tb_pool:
                        csc = tb_pool.tile([128, 128], F16, tag="csc")
                        nc.vector.memset(csc[:, 0:1], 1.0)
                        nc.vector.memset(csc[:, 1:128], 0.0)
                        for g in range(NG):
                            nc.sync.dma_start(
                                out=dt_T3.ap()[512 * g:512 * (g + 1), 0:D]
                                    .rearrange("(c p) w -> p c w", p=128),
                                in_=dt_h16g.ap()[:, 512 * g:512 * (g + 1)]
                                    .rearrange("p (c d) -> p c d", c=4))
                            nc.sync.dma_start(
                                out=dt_T3.ap()[512 * g:512 * (g + 1), D:TROW]
                                    .rearrange("(c p) w -> p c w", p=128),
                                in_=csc[:, None, :].to_broadcast([128, 4, 128]))

                    # ---------- batched node finish ----------
                    with tc.tile_pool(name="nfin", bufs=1) as nf:
                        s1N = sNA[:, :, 0:12:3]
                        suhN = sNA[:, :, 1:12:3]
                        sutN = sNA[:, :, 2:12:3]
                        s2N = sNA[:, :, 12:16]
                        mu = nf.tile([128, NPK], F32)
                        nc.vector.tensor_copy(out=mu[:].rearrange("p (g c) -> p g c", g=NG), in_=s1N)
                        nc.vector.tensor_scalar_mul(out=mu[:], in0=mu[:], scalar1=1.0 / 128.0)
                        t0 = nf.tile([128, NPK], F32)
                        nc.vector.tensor_mul(out=t0[:], in0=mu[:], in1=mu[:])
                        nc.vector.tensor_scalar_mul(out=t0[:], in0=t0[:], scalar1=-128.0)
                        nc.vector.tensor_tensor(out=t0[:].rearrange("p (g c) -> p g c", g=NG),
                                                in0=t0[:].rearrange("p (g c) -> p g c", g=NG),
                                                in1=s2N, op=OP.add)
                        rstd = nf.tile([128, NPK], F32)
                        nc.scalar.activation(out=rstd[:], in_=t0[:], func=AF.Sqrt, scale=1.0 / 127.0)
                        nc.vector.tensor_scalar_add(out=rstd[:], in0=rstd[:], scalar1=EPS)
                        nc.vector.reciprocal(out=rstd[:], in_=rstd[:])
                        for suT, sidx, cidx, dest in ((suhN, 0, 3, eh16), (sutN, 1, 4, et16)):
                            m1 = nf.tile([128, NPK], F32, tag="m1")
                            nc.vector.tensor_scalar_mul(out=m1[:], in0=mu[:], scalar1=cst[:, sidx:sidx + 1])
                            nc.vector.tensor_tensor(out=m1[:].rearrange("p (g c) -> p g c", g=NG),
                                                    in0=suT,
                                                    in1=m1[:].rearrange("p (g c) -> p g c", g=NG),
                                                    op=OP.subtract)
                            nc.vector.tensor_mul(out=m1[:], in0=m1[:], in1=rstd[:])
                            nc.vector.tensor_scalar_add(out=m1[:], in0=m1[:], scalar1=cst[:, cidx:cidx + 1])
                            nc.scalar.activation(out=dest[:], in_=m1[:], func=AF.Tanh)

                    # ---------- T3 eh column (one strided DMA, 2B runs) ----
                    nc.sync.dma_start(
                        out=dt_T3.ap()[:, D + 1:D + 2]
                            .rearrange("(g c p) w -> p (g c) w", p=128, c=4),
                        in_=eh16[:, :, None])

                    # ---- loops A+B: er stats in 3 chunks, gathers stream, fc inline ----
                    CH = [(0, 7), (7, 14), (14, NB_U)]
                    with tc.tile_pool(name="awork", bufs=2) as aw, \
                         tc.tile_pool(name="bwork", bufs=2) as bw, \
                         tc.tile_pool(name="tgp", bufs=3) as tgp, \
                         tc.tile_pool(name="ohplp", bufs=3) as ohp, \
                         tc.tile_pool(name="bsc", bufs=3) as bsc, \
                         tc.tile_pool(name="efin", bufs=2) as ef, \
                         tc.tile_pool(name="apsum", bufs=2, space="PSUM") as ap_, \
                         tc.tile_pool(name="bps1", bufs=2, space="PSUM") as bp1, \
                         tc.tile_pool(name="bps2", bufs=2, space="PSUM") as bp2:
                        # ---- loop A (chunked) + batched er finishes per chunk ----
                        for lo, hi in CH:
                            for b in range(lo, hi):
                                rTb = aw.tile([128, e_blk], F16, tag="rTb")
                                nc.scalar.dma_start(out=rTb[:], in_=dt_rT.ap()[:, b * e_blk:(b + 1) * e_blk])
                                ohtb = aw.tile([128, e_blk], F16, tag="ohtb")
                                nc.scalar.dma_start(out=ohtb[:], in_=dt_oht.ap()[:, b * e_blk:(b + 1) * e_blk])
                                psED = ap_.tile([128, 4 * t_b], F32, tag="psED")
                                for t in range(t_b):
                                    nc.tensor.matmul(psED[:, 2 * t:2 * t + 2],
                                                     rTb[:, 128 * t:128 * (t + 1)],
                                                     wr_sb[:], start=True, stop=True)
                                    nc.tensor.matmul(psED[:, 3 * t_b + t:3 * t_b + t + 1],
                                                     ohtb[:, 128 * t:128 * (t + 1)],
                                                     et16[:, b:b + 1], start=True, stop=True)
                                # square in place (serializes with the s1/su reads above)
                                if b % 2 == 0:
                                    nc.scalar.activation(out=rTb[:], in_=rTb[:], func=AF.Square)
                                else:
                                    nc.vector.tensor_mul(out=rTb[:], in0=rTb[:], in1=rTb[:])
                                for t in range(t_b):
                                    nc.tensor.matmul(psED[:, 2 * t_b + t:2 * t_b + t + 1],
                                                     rTb[:, 128 * t:128 * (t + 1)],
                                                     ones_sb[:], start=True, stop=True)
                                nc.vector.tensor_copy(out=s1A[:, b * t_b:(b + 1) * t_b], in_=psED[:, 0:2 * t_b:2])
                                nc.vector.tensor_copy(out=suA[:, b * t_b:(b + 1) * t_b], in_=psED[:, 1:2 * t_b:2])
                                nc.vector.tensor_copy(out=s2A[:, b * t_b:(b + 1) * t_b], in_=psED[:, 2 * t_b:3 * t_b])
                                nc.vector.tensor_copy(out=etdA[:, b * t_b:(b + 1) * t_b], in_=psED[:, 3 * t_b:4 * t_b])
                            # batched er finish for this chunk
                            c0, cn = lo * t_b, (hi - lo) * t_b
                            mu = ef.tile([128, 7 * t_b], F32, tag="fmu")
                            muv = mu[:, 0:cn]
                            nc.vector.tensor_scalar_mul(out=muv, in0=s1A[:, c0:c0 + cn], scalar1=1.0 / 128.0)
                            t0 = ef.tile([128, 7 * t_b], F32, tag="ft0")
                            t0v = t0[:, 0:cn]
                            nc.vector.tensor_mul(out=t0v, in0=muv, in1=muv)
                            nc.vector.tensor_scalar_mul(out=t0v, in0=t0v, scalar1=-128.0)
                            nc.vector.tensor_add(out=t0v, in0=t0v, in1=s2A[:, c0:c0 + cn])
                            nc.scalar.activation(out=t0v, in_=t0v, func=AF.Sqrt, scale=1.0 / 127.0)
                            nc.vector.tensor_scalar_add(out=t0v, in0=t0v, scalar1=EPS)
                            nc.vector.reciprocal(out=t0v, in_=t0v)
                            nc.vector.tensor_scalar_mul(out=muv, in0=muv, scalar1=cst[:, 2:3])
                            nc.vector.tensor_sub(out=muv, in0=suA[:, c0:c0 + cn], in1=muv)
                            nc.vector.tensor_mul(out=muv, in0=muv, in1=t0v)
                            nc.vector.tensor_scalar_add(out=muv, in0=muv, scalar1=cst[:, 5:6])
                            nc.scalar.activation(out=erpA[:, c0:c0 + cn], in_=muv, func=AF.Tanh)

                        # ---- loop B: gather -> ex -> scatter -> fc (inline) ----
                        for b in range(NB_U):
                            ohpl = ohp.tile([128, t_b, 128], F16, tag="ohpl")
                            nc.vector.tensor_tensor(
                                out=ohpl[:, :, :],
                                in0=iota_sb[:, None, :].to_broadcast([128, t_b, 128]),
                                in1=dstl_sb[:, b * t_b:(b + 1) * t_b, None].to_broadcast([128, t_b, 128]),
                                op=OP.is_equal)
                            tg = tgp.tile([128, t_b, TROW], F16, tag="tg")
                            nc.gpsimd.dma_gather(
                                out_ap=tg[:, :, :], in_ap=dt_T3.ap(),
                                idxs_ap=isrc_sb[:, b * s_b:(b + 1) * s_b],
                                num_idxs=e_blk, num_idxs_reg=e_blk, elem_size=TROW,
                                single_packet=False)
                            ehs = bsc.tile([128, t_b], F32, tag="ehs")
                            nc.scalar.activation(out=ehs[:], in_=tg[:, :, D + 1], func=AF.Copy)
                            ex = bsc.tile([128, t_b], F32, tag="ex")
                            nc.vector.tensor_add(out=ex[:], in0=ehs[:], in1=etdA[:, b * t_b:(b + 1) * t_b])
                            nc.vector.tensor_add(out=ex[:], in0=ex[:], in1=erpA[:, b * t_b:(b + 1) * t_b])
                            nc.scalar.activation(out=ex[:], in_=ex[:], func=AF.Exp)
                            nc.vector.tensor_scalar(out=ex[:], in0=ex[:], scalar1=1.0,
                                                    scalar2=None, op0=OP.max)
                            tgs = bw.tile([128, t_b, 130], F16, tag="tgs")
                            nc.vector.tensor_tensor(
                                out=tgs[:, :, :], in0=tg[:, :, 0:130],
                                in1=ex[:, :, None].to_broadcast([128, t_b, 130]),
                                op=OP.mult)
                            psF = bp1.tile([128, 129], F32, tag="psF")
                            for t in range(t_b):
                                nc.tensor.matmul(psF[:], ohpl[:, t, :], tgs[:, t, 0:129],
                                                 start=(t == 0), stop=(t == t_b - 1))
                            featst = bsc.tile([128, 129], F32, tag="featst")
                            nc.scalar.activation(out=featst[:], in_=psF[:], func=AF.Copy)
                            # inline fc for this block
                            esc = bsc.tile([128, 1], F32, tag="esc")
                            nc.vector.tensor_scalar(out=esc[:], in0=featst[:, 128:129],
                                                    scalar1=1e-30, scalar2=None, op0=OP.max)
                            nc.vector.reciprocal(out=esc[:], in_=esc[:])
                            fs = bsc.tile([128, 128], F16, tag="fs")
                            nc.vector.tensor_scalar_mul(out=fs[:], in0=featst[:, 0:128], scalar1=esc[:, :])
                            psT = bp2.tile([128, 128], F16, tag="psT")
                            nc.tensor.transpose(psT[:], fs[:], ident_sb[:])
                            fT = bsc.tile([128, 128], F16, tag="fT")
                            nc.vector.tensor_copy(out=fT[:], in_=psT[:])
                            psO = bp2.tile([128, 128], F32, tag="psO")
                            nc.tensor.matmul(psO[:], fT[:], fcw_sb[:], start=True, stop=True)
                            nc.vector.tensor_add(out=obA[:, b, :], in0=psO[:], in1=fcb_sb[:])

                    # ---------- batched L2 normalize + output ----------
                    with tc.tile_pool(name="fin", bufs=1) as fin:
                        scr = fin.tile([128, NB_U, 128], F32)
                        nc.vector.tensor_mul(out=scr[:], in0=obA[:], in1=obA[:])
                        nrm = fin.tile([128, NB_U], F32)
                        nc.vector.tensor_reduce(out=nrm[:], in_=scr[:],
                                                axis=mybir.AxisListType.X, op=OP.add)
                        nc.scalar.activation(out=nrm[:], in_=nrm[:], func=AF.Sqrt)
                        nc.vector.tensor_scalar(out=nrm[:], in0=nrm[:], scalar1=1e-12,
                                                scalar2=None, op0=OP.max)
                        nc.vector.reciprocal(out=nrm[:], in_=nrm[:])
                        obn = fin.tile([128, NB_U, 128], F32)
                        nc.vector.tensor_tensor(
                            out=obn[:], in0=obA[:],
                            in1=nrm[:, :, None].to_broadcast([128, NB_U, 128]),
                            op=OP.mult)
                        nc.sync.dma_start(
                            out=dt_out.ap().rearrange("(b p) d -> p b d", p=128),
                            in_=obn[:])

            if loop_k == 1:
                loop_body()
            else:
                with tc.For_i(0, loop_k, 1):
                    loop_body()

    nc.compile()
    if for_hw:
        nc.m = get_hw_module(nc.m)
    return nc


# ------------------------------------------------------------------- runner
class Runner:
    def __init__(self, nc, n_cores=NCORES):
        import jax
        from concourse.bass2jax import (_bass_exec_p, partition_id_tensor,
                                        install_neuronx_cc_hook)
        from jax.sharding import Mesh, PartitionSpec, NamedSharding
        from jax.experimental.shard_map import shard_map
        install_neuronx_cc_hook()
        self.jax = jax
        self.n_cores = n_cores
        pname = nc.partition_id_tensor.name if nc.partition_id_tensor else None
        in_names, out_names, out_avals = [], [], []
        for alloc in nc.m.functions[0].allocations:
            if not isinstance(alloc, mybir.MemoryLocationSet):
                continue
            name = alloc.memorylocations[0].name
            if alloc.kind == "ExternalInput":
                if name != pname:
                    in_names.append(name)
            elif alloc.kind == "ExternalOutput":
                out_names.append(name)
                out_avals.append(jax.core.ShapedArray(
                    tuple(alloc.tensor_shape), mybir.dt.np(alloc.dtype)))
        self.in_names, self.out_names, self.out_avals = in_names, out_names, out_avals
        n_params = len(in_names)
        all_in = list(in_names) + list(out_names)
        if pname is not None:
            all_in.append(pname)

        def _body(*args):
            operands = list(args)
            if pname is not None:
                operands.append(partition_id_tensor())
            return tuple(_bass_exec_p.bind(
                *operands, out_avals=tuple(out_avals), in_names=tuple(all_in),
                out_names=tuple(out_names), lowering_input_output_aliases=(),
                sim_require_finite=True, sim_require_nnan=True, nc=nc))

        devices = jax.devices()[:n_cores]
        self.mesh = Mesh(np.asarray(devices), ("core",))
        self.sharding = NamedSharding(self.mesh, PartitionSpec("core"))
        donate = tuple(range(n_params, n_params + len(out_names)))
        self.fn = jax.jit(shard_map(
            _body, mesh=self.mesh,
            in_specs=(PartitionSpec("core"),) * (n_params + len(out_names)),
            out_specs=(PartitionSpec("core"),) * len(out_names),
            check_rep=False), donate_argnums=donate, keep_unused=True)

    def put_inputs(self, in_maps):
        return [self.jax.device_put(
            np.concatenate([np.asarray(in_maps[c][nm]) for c in range(self.n_cores)], axis=0),
            self.sharding) for nm in self.in_names]

    def put_zeros(self):
        return [self.jax.device_put(
            np.zeros((self.n_cores * a.shape[0], *a.shape[1:]), a.dtype), self.sharding)
            for a in self.out_avals]

    def run(self, dev_in, dev_zeros):
        outs = self.fn(*dev_in, *dev_zeros)
        self.jax.block_until_ready(outs)
        return outs

    def unpack(self, outs):
        return [{nm: np.asarray(outs[i]).reshape(self.n_cores, *self.out_avals[i].shape)[c]
                 for i, nm in enumerate(self.out_names)} for c in range(self.n_cores)]


_CACHE = {}


def _get_runner(t_b, loop_k=1):
    key = (t_b, loop_k)
    if key not in _CACHE:
        nc = build_program(t_b, loop_k)
        _CACHE[key] = Runner(nc)
    return _CACHE[key]


def kernel(**inputs):
    in_maps, meta = _host_prep(**inputs)
    r = _get_runner(meta["t_b"], 1)
    dev = r.put_inputs(in_maps)
    res = r.unpack(r.run(dev, r.put_zeros()))
    out = np.empty((N, D), np.float32)
    for k in range(NCORES):
        nlo = k * NB_NODES
        nhi = min(nlo + NB_NODES, N)
        out[nlo:nhi] = res[k]["out"][:nhi - nlo]
    return out
